# revision 40
# baseline (speedup 1.0000x reference)
"""MetaRoPE kernel for Trainium2, 8 NeuronCores — fp16 I/O + DVE 2x-mode.

Reference computation:
    r = rotate_m[token_positions]            # [S, D, D], block-diag 2x2 rotations
    out = einsum('bhsi,soi->bhso', x, r)     # x: [4, 32, 4096, 64] fp32

Because r is block-diagonal with 2x2 blocks, for each position s and pair k:
    out[2k]   = a*x[2k] + b*x[2k+1]     (a = r[2k,2k],   b = r[2k,2k+1])
    out[2k+1] = c*x[2k+1] + d*x[2k]     (c = r[2k+1,2k+1], d = r[2k+1,2k])
which we compute elementwise as
    out = x * A + pairswap(x * B')
with host-precomputed tables A, B' of shape [S, D]:
    A[s,2k] = a, A[s,2k+1] = c
    B'[s,2k] = d, B'[s,2k+1] = b       (B' is pre-pairswapped so that
                                        pairswap(x*B') lands b*x_odd on even
                                        lanes and d*x_even on odd lanes)

Precision/bandwidth: the correctness gate is rel_err < 2e-2; fp16 end-to-end
(host converts x fp32->fp16, device computes in fp16, host converts the fp16
result back) measures ~1.1e-3 and halves both HBM traffic and DVE element
cost vs fp32. Plain InstTensorTensor ops hit the DVE 2x_1p perf mode with
packed fp16 (~0.5 ns/elem/partition measured, including the stride -1
pair-swap operand). Notes from measurement on HW:
  - scalar_tensor_tensor (fused 3-input op) supports NO DVE perf modes and
    runs ~1.2 ns/elem — slower than two plain ops.
  - 4-dim merged APs (one mul writing u and o via a broadcast x) run ~15%
    slower per element than 3-dim APs.
  - GpSimd tensor ops are Q7 software (~2.5-6.4 ns/elem) AND degrade
    concurrent DVE throughput via SBUF contention — never offload to it.
  - DMA floor is ~17.9 MB/core at ~360 GB/s ~= 50 us, under the DVE's ~64 us
    busy (48 us compute + sem/dispatch overhead), so DVE is the roofline.

Sharding: x reshaped to [128 (b,h) slabs, 4096, 64]; 16 slabs per core.
Each slab [4096*64] is viewed as [128 partitions, 2048 free] (contiguous per
partition; partition p holds positions 32p..32p+31). Tables are replicated
to every core as [128, 2048] fp16 tiles that match that layout for every
slab.

Per core the 16 slabs are processed in chunks (CHUNK_PLAN, tapered small at
the ends to shrink pipeline ramp/tail). Each chunk: one load (HWDGE on the
sync ring), two DVE tensor_muls (tables broadcast across the chunk's slabs
via a step-0 AP dim) + one pair-swapped in-place tensor_add, one store
(HWDGE on the scalar ring). Table halves split across rings: first halves
on the scalar ring up front, second halves on the sync ring emitted after
the head chunk's first half-compute, so that compute (which needs only
table cols [0:1024)) starts ~3.5 us earlier — tile deps follow emission
order, and every DVE op emitted after a DMA to tb/ta waits on it.

Measured: 71.8-73.7 us HW exec across runs (from the 149.8 us fp32
baseline), rel err 1.15e-3. Breakdown at this point: ~6.9 us fixed engine
init before the first DMA trigger, ~64.3 us DVE busy (51.2 us fp16-2x
compute roofline + ~8.1 us semaphore slices + ~1.3 us engine TENSOR_LOAD +
per-instr overhead ~63 ns x 38), ~1 us tail. The semaphore slice count
(~65) is framework scaffolding — invariant to chunking or buffering
(single-buffered x/out vs pooled tiles measured identical).
"""

import sys

import numpy as np

_TRN_REPO = "/opt/trn_rl_repo"
if _TRN_REPO not in sys.path:
    sys.path.insert(0, _TRN_REPO)

B, H, S, D = 4, 32, 4096, 64
BH = B * H                      # 128 (b,h) slabs
N_CORES = 8
BH_PER_CORE = BH // N_CORES     # 16 slabs per core
FREE = (S // 128) * D           # 2048 free elements per partition per slab
ROWS = BH_PER_CORE * 128        # 2048 dram rows per core, [ROWS, FREE] fp16
# slabs per chunk, tapered: small first chunk so compute starts early,
# small last chunk so the final store is short
CHUNK_PLAN = [1, 1, 2, 2, 2, 2, 2, 2, 1, 1]
assert sum(CHUNK_PLAN) == BH_PER_CORE
U_BUFS = 3

_prog_cache = {}


def _build_program():
    """Build (and cache) the SPMD Bass program for one core."""
    if "nc" in _prog_cache:
        return _prog_cache["nc"]

    import concourse.bacc as bacc
    import concourse.bass as bass
    import concourse.mybir as mybir
    import concourse.tile as tile

    f16 = mybir.dt.float16
    nc = bacc.Bacc(
        "TRN2", target_bir_lowering=False, debug=False, num_devices=N_CORES
    )
    x_d = nc.dram_tensor("x", [ROWS, FREE], f16, kind="ExternalInput").ap()
    ta_d = nc.dram_tensor("ta", [128, FREE], f16, kind="ExternalInput").ap()
    tb_d = nc.dram_tensor("tb", [128, FREE], f16, kind="ExternalInput").ap()
    o_d = nc.dram_tensor("out", [ROWS, FREE], f16, kind="ExternalOutput").ap()

    with tile.TileContext(nc) as tc:
        with (
            tc.tile_pool(name="tabs", bufs=1) as tabs,
            tc.tile_pool(name="xbig", bufs=1) as xbigp,
            tc.tile_pool(name="u", bufs=U_BUFS) as upool,
            tc.tile_pool(name="obig", bufs=1) as obigp,
        ):
            # table loads go on the scalar HWDGE ring (idle at start) so
            # they overlap the first x-chunk load on the sync ring; halves
            # ordered so the first half-slab compute (needs tb+ta cols
            # [0:hf)) can start before the full tables land
            tb = tabs.tile([128, FREE], f16)
            ta = tabs.tile([128, FREE], f16)
            hf = FREE // 2
            nc.scalar.dma_start(tb[:, :hf], tb_d[:, :hf])
            nc.scalar.dma_start(ta[:, :hf], ta_d[:, :hf])
            # second halves are loaded from inside the chunk loop (on the
            # sync ring, after the first x chunk) so the head chunk's first
            # half-compute — emitted before them — only depends on the
            # first-half table loads (tile deps follow emission order).
            # (Tried instead putting ta's first half on the sync ring ahead
            # of x: per-queue transfer serialization pushed the first x half
            # later and the first multiply slipped ~2 us — keep both first
            # halves on the scalar ring.)

            # x and out live in single whole-core SBUF buffers (64 KiB per
            # partition each): every slice is written once and read once, so
            # there are no tile-reuse WAR waits — the only semaphores left
            # are load-done -> mul and add-done -> store per chunk
            xbig = xbigp.tile([128, BH_PER_CORE * FREE], f16)
            obig = obigp.tile([128, BH_PER_CORE * FREE], f16)

            def compute(xoff, ut, nsl, lo, sz):
                """u = x*tb; o = x*ta; o += pairswap(u) on cols [lo, lo+sz)
                of each of the nsl slabs at element offset xoff in the big
                x/out buffers (3-dim APs throughout)."""
                if nsl == 1:
                    xs = xbig[:, xoff + lo : xoff + lo + sz]
                    us = ut[:, lo : lo + sz]
                    os_ = obig[:, xoff + lo : xoff + lo + sz]
                    nc.vector.tensor_mul(us, xs, tb[:, lo : lo + sz])
                    nc.vector.tensor_mul(os_, xs, ta[:, lo : lo + sz])
                else:
                    assert lo == 0 and sz == nsl * FREE
                    cf = nsl * FREE
                    x3 = xbig[:, xoff : xoff + cf].rearrange(
                        "p (j f) -> p j f", j=nsl
                    )
                    u3 = ut[:].rearrange("p (j f) -> p j f", j=nsl)
                    os_ = obig[:, xoff : xoff + cf]
                    o3 = os_.rearrange("p (j f) -> p j f", j=nsl)
                    ta_b = bass.AP(
                        ta[:].tensor, ta[:].offset,
                        [ta[:].ap[0], [0, nsl], ta[:].ap[1]],
                    )
                    tb_b = bass.AP(
                        tb[:].tensor, tb[:].offset,
                        [tb[:].ap[0], [0, nsl], tb[:].ap[1]],
                    )
                    nc.vector.tensor_mul(u3, x3, tb_b)
                    nc.vector.tensor_mul(o3, x3, ta_b)
                    us = ut[:]
                usw = us.rearrange("p (n two) -> p n two", two=2)[:, :, ::-1]
                os3 = os_.rearrange("p (n two) -> p n two", two=2)
                nc.vector.tensor_add(os3, os3, usw)

            row0 = 0
            for ci, nsl in enumerate(CHUNK_PLAN):
                first = ci == 0
                last = ci == len(CHUNK_PLAN) - 1
                cfree = nsl * FREE
                xoff = row0 * FREE
                rows = x_d[row0 * 128 : (row0 + nsl) * 128, :]
                xts = xbig[:, xoff : xoff + cfree]
                if first:
                    # split the first load so compute can start after 0.25 MiB
                    assert nsl == 1
                    h = cfree // 2
                    nc.sync.dma_start(xts[:, :h], rows[:, :h])
                    nc.sync.dma_start(xts[:, h:], rows[:, h:])
                else:
                    src = rows.rearrange("(j p) f -> p j f", j=nsl)
                    nc.sync.dma_start(
                        xts.rearrange("p (j f) -> p j f", j=nsl), src
                    )

                ut = upool.tile([128, cfree], f16, tag="u")
                ots = obig[:, xoff : xoff + cfree]
                orows = o_d[row0 * 128 : (row0 + nsl) * 128, :]

                if first or last:
                    # head chunk in halves: starts computing after the first
                    # half-load. Tail chunk in quarters: the final store is
                    # only 0.125 MB, shrinking the end-of-run barrier wait
                    # for it (~1 us off the measured span).
                    nparts = 2 if first else 4
                    h = cfree // nparts
                    for hi in range(nparts):
                        compute(xoff, ut, 1, hi * h, h)
                        if first and hi == 0:
                            # second table halves, after the head's first
                            # half-compute in emission order
                            nc.sync.dma_start(tb[:, hf:], tb_d[:, hf:])
                            nc.sync.dma_start(ta[:, hf:], ta_d[:, hf:])
                        nc.scalar.dma_start(
                            orows[:, hi * h : (hi + 1) * h],
                            ots[:, hi * h : (hi + 1) * h],
                        )
                else:
                    compute(xoff, ut, nsl, 0, cfree)
                    dst = orows.rearrange("(j p) f -> p j f", j=nsl)
                    nc.scalar.dma_start(
                        dst, ots.rearrange("p (j f) -> p j f", j=nsl)
                    )
                row0 += nsl

    nc.compile()
    _prog_cache["nc"] = nc
    return nc


def _default_rotate_m(theta=10000.0):
    """Rebuild the reference's rotation buffer if the harness doesn't pass it."""
    half = D // 2
    try:  # replicate the reference's jax-f32 arithmetic exactly if possible
        import jax.numpy as jnp

        pos = np.asarray(jnp.arange(S, dtype=jnp.float32))
        inv_freq = np.asarray(
            theta ** (-(2.0 * jnp.arange(half, dtype=jnp.float32)) / D)
        )
        ang = np.asarray(pos[:, None] * inv_freq[None, :], dtype=np.float32)
        c, s = np.asarray(jnp.cos(ang)), np.asarray(jnp.sin(ang))
    except Exception:
        pos = np.arange(S, dtype=np.float32)
        exp = (-(2.0 * np.arange(half, dtype=np.float32)) / D).astype(np.float32)
        inv_freq = np.power(np.float32(theta), exp, dtype=np.float32)
        ang = (pos[:, None] * inv_freq[None, :]).astype(np.float32)
        c, s = np.cos(ang, dtype=np.float32), np.sin(ang, dtype=np.float32)
    idx = 2 * np.arange(half)
    r = np.zeros((S, D, D), dtype=np.float32)
    r[:, idx, idx] = c
    r[:, idx, idx + 1] = -s
    r[:, idx + 1, idx] = s
    r[:, idx + 1, idx + 1] = c
    return r


def _tables(token_positions, rotate_m):
    """Host-precompute the [128, FREE] fp16 A and B' tables (see docstring)."""
    if rotate_m is None:
        rotate_m = _default_rotate_m()
    r = np.asarray(rotate_m, dtype=np.float32)[np.asarray(token_positions)]
    idx = np.arange(D // 2) * 2
    a = r[:, idx, idx]            # x_even -> out_even
    b = r[:, idx, idx + 1]        # x_odd  -> out_even
    c = r[:, idx + 1, idx + 1]    # x_odd  -> out_odd
    d = r[:, idx + 1, idx]        # x_even -> out_odd
    A = np.empty((S, D), np.float32)
    A[:, 0::2] = a
    A[:, 1::2] = c
    Bp = np.empty((S, D), np.float32)
    Bp[:, 0::2] = d
    Bp[:, 1::2] = b
    return (
        np.ascontiguousarray(A.reshape(128, FREE)).astype(np.float16),
        np.ascontiguousarray(Bp.reshape(128, FREE)).astype(np.float16),
    )


def _in_maps(x, token_positions, rotate_m):
    ta, tb = _tables(token_positions, rotate_m)
    xs = np.asarray(x, dtype=np.float32).astype(np.float16).reshape(
        N_CORES, ROWS, FREE
    )
    xs = np.ascontiguousarray(xs)
    return [{"x": xs[i], "ta": ta, "tb": tb} for i in range(N_CORES)]


def _run(x, token_positions, rotate_m=None, trace=False, trace_cores=None):
    from concourse.bass_utils import run_bass_kernel_spmd

    nc = _build_program()
    in_maps = _in_maps(x, token_positions, rotate_m)
    res = run_bass_kernel_spmd(
        nc,
        in_maps,
        list(range(N_CORES)),
        trace=trace,
        trace_cores=trace_cores,
    )
    out = np.concatenate(
        [res.results[i]["out"].reshape(1, ROWS * FREE) for i in range(N_CORES)]
    ).reshape(B, H, S, D).astype(np.float32)
    return out, res


def kernel(x, token_positions, rotate_m=None, **_unused):
    out, _ = _run(x, token_positions, rotate_m, trace=False)
    return out


# revision 42
# speedup vs baseline: 1.0072x; 1.0072x over previous
"""MetaRoPE kernel for Trainium2, 8 NeuronCores — fp16 I/O + DVE 2x-mode.

Reference computation:
    r = rotate_m[token_positions]            # [S, D, D], block-diag 2x2 rotations
    out = einsum('bhsi,soi->bhso', x, r)     # x: [4, 32, 4096, 64] fp32

Because r is block-diagonal with 2x2 blocks, for each position s and pair k:
    out[2k]   = a*x[2k] + b*x[2k+1]     (a = r[2k,2k],   b = r[2k,2k+1])
    out[2k+1] = c*x[2k+1] + d*x[2k]     (c = r[2k+1,2k+1], d = r[2k+1,2k])
which we compute elementwise as
    out = x * A + pairswap(x * B')
with host-precomputed tables A, B' of shape [S, D]:
    A[s,2k] = a, A[s,2k+1] = c
    B'[s,2k] = d, B'[s,2k+1] = b       (B' is pre-pairswapped so that
                                        pairswap(x*B') lands b*x_odd on even
                                        lanes and d*x_even on odd lanes)

Precision/bandwidth: the correctness gate is rel_err < 2e-2; fp16 end-to-end
(host converts x fp32->fp16, device computes in fp16, host converts the fp16
result back) measures ~1.1e-3 and halves both HBM traffic and DVE element
cost vs fp32. Plain InstTensorTensor ops hit the DVE 2x_1p perf mode with
packed fp16 (~0.5 ns/elem/partition measured, including the stride -1
pair-swap operand). Notes from measurement on HW:
  - scalar_tensor_tensor (fused 3-input op) supports NO DVE perf modes and
    runs ~1.2 ns/elem — slower than two plain ops.
  - 4-dim merged APs (one mul writing u and o via a broadcast x) run ~15%
    slower per element than 3-dim APs.
  - GpSimd tensor ops are Q7 software (~2.5-6.4 ns/elem) AND degrade
    concurrent DVE throughput via SBUF contention — never offload to it.
  - DMA floor is ~17.9 MB/core at ~360 GB/s ~= 50 us, under the DVE's ~64 us
    busy (48 us compute + sem/dispatch overhead), so DVE is the roofline.

Sharding: x reshaped to [128 (b,h) slabs, 4096, 64]; 16 slabs per core.
Each slab [4096*64] is viewed as [128 partitions, 2048 free] (contiguous per
partition; partition p holds positions 32p..32p+31). Tables are replicated
to every core as [128, 2048] fp16 tiles that match that layout for every
slab.

Per core the 16 slabs are processed in chunks (CHUNK_PLAN, tapered small at
the ends to shrink pipeline ramp/tail). Each chunk: one load (HWDGE on the
sync ring), two DVE tensor_muls (tables broadcast across the chunk's slabs
via a step-0 AP dim) + one pair-swapped in-place tensor_add, one store
(HWDGE on the scalar ring). Table halves split across rings: first halves
on the scalar ring up front, second halves on the sync ring emitted after
the head chunk's first half-compute, so that compute (which needs only
table cols [0:1024)) starts ~3.5 us earlier — tile deps follow emission
order, and every DVE op emitted after a DMA to tb/ta waits on it.

Measured: 71.8-73.7 us HW exec across runs (from the 149.8 us fp32
baseline), rel err 1.15e-3. Breakdown at this point: ~6.9 us fixed engine
init before the first DMA trigger, ~64.3 us DVE busy (51.2 us fp16-2x
compute roofline + ~8.1 us semaphore slices + ~1.3 us engine TENSOR_LOAD +
per-instr overhead ~63 ns x 38), ~1 us tail. The semaphore slice count
(~65) is framework scaffolding — invariant to chunking or buffering
(single-buffered x/out vs pooled tiles measured identical).
"""

import sys

import numpy as np

_TRN_REPO = "/opt/trn_rl_repo"
if _TRN_REPO not in sys.path:
    sys.path.insert(0, _TRN_REPO)

B, H, S, D = 4, 32, 4096, 64
BH = B * H                      # 128 (b,h) slabs
N_CORES = 8
BH_PER_CORE = BH // N_CORES     # 16 slabs per core
FREE = (S // 128) * D           # 2048 free elements per partition per slab
ROWS = BH_PER_CORE * 128        # 2048 dram rows per core, [ROWS, FREE] fp16
# slabs per chunk, tapered: small first chunk so compute starts early,
# small last chunk so the final store is short
CHUNK_PLAN = [1, 1, 2, 2, 2, 2, 2, 2, 1, 1]
assert sum(CHUNK_PLAN) == BH_PER_CORE
U_BUFS = 3

_prog_cache = {}


def _build_program():
    """Build (and cache) the SPMD Bass program for one core."""
    if "nc" in _prog_cache:
        return _prog_cache["nc"]

    import concourse.bacc as bacc
    import concourse.bass as bass
    import concourse.mybir as mybir
    import concourse.tile as tile

    f16 = mybir.dt.float16
    nc = bacc.Bacc(
        "TRN2", target_bir_lowering=False, debug=False, num_devices=N_CORES
    )
    x_d = nc.dram_tensor("x", [ROWS, FREE], f16, kind="ExternalInput").ap()
    ta_d = nc.dram_tensor("ta", [128, FREE], f16, kind="ExternalInput").ap()
    tb_d = nc.dram_tensor("tb", [128, FREE], f16, kind="ExternalInput").ap()
    o_d = nc.dram_tensor("out", [ROWS, FREE], f16, kind="ExternalOutput").ap()

    with tile.TileContext(nc) as tc:
        with (
            tc.tile_pool(name="tabs", bufs=1) as tabs,
            tc.tile_pool(name="xbig", bufs=1) as xbigp,
            tc.tile_pool(name="u", bufs=U_BUFS) as upool,
            tc.tile_pool(name="obig", bufs=1) as obigp,
        ):
            # table loads go on the scalar HWDGE ring (idle at start) so
            # they overlap the first x-chunk load on the sync ring; halves
            # ordered so the first half-slab compute (needs tb+ta cols
            # [0:hf)) can start before the full tables land
            tb = tabs.tile([128, FREE], f16)
            ta = tabs.tile([128, FREE], f16)
            hf = FREE // 2
            qf = FREE // 4
            # first halves in quarter pieces: the head chunk's first
            # quarter-compute needs only cols [0:qf) of each table, so it
            # gates on one 0.25 MB piece per table instead of 0.5 MB
            nc.scalar.dma_start(tb[:, :qf], tb_d[:, :qf])
            nc.scalar.dma_start(ta[:, :qf], ta_d[:, :qf])
            nc.scalar.dma_start(tb[:, qf:hf], tb_d[:, qf:hf])
            nc.scalar.dma_start(ta[:, qf:hf], ta_d[:, qf:hf])
            # second halves are loaded from inside the chunk loop (on the
            # sync ring, after the first x chunk) so the head chunk's first
            # half-compute — emitted before them — only depends on the
            # first-half table loads (tile deps follow emission order).
            # (Tried instead putting ta's first half on the sync ring ahead
            # of x: per-queue transfer serialization pushed the first x half
            # later and the first multiply slipped ~2 us — keep both first
            # halves on the scalar ring.)

            # x and out live in single whole-core SBUF buffers (64 KiB per
            # partition each): every slice is written once and read once, so
            # there are no tile-reuse WAR waits — the only semaphores left
            # are load-done -> mul and add-done -> store per chunk
            xbig = xbigp.tile([128, BH_PER_CORE * FREE], f16)
            obig = obigp.tile([128, BH_PER_CORE * FREE], f16)

            def compute(xoff, ut, nsl, lo, sz):
                """u = x*tb; o = x*ta; o += pairswap(u) on cols [lo, lo+sz)
                of each of the nsl slabs at element offset xoff in the big
                x/out buffers (3-dim APs throughout)."""
                if nsl == 1:
                    xs = xbig[:, xoff + lo : xoff + lo + sz]
                    us = ut[:, lo : lo + sz]
                    os_ = obig[:, xoff + lo : xoff + lo + sz]
                    nc.vector.tensor_mul(us, xs, tb[:, lo : lo + sz])
                    nc.vector.tensor_mul(os_, xs, ta[:, lo : lo + sz])
                else:
                    assert lo == 0 and sz == nsl * FREE
                    cf = nsl * FREE
                    x3 = xbig[:, xoff : xoff + cf].rearrange(
                        "p (j f) -> p j f", j=nsl
                    )
                    u3 = ut[:].rearrange("p (j f) -> p j f", j=nsl)
                    os_ = obig[:, xoff : xoff + cf]
                    o3 = os_.rearrange("p (j f) -> p j f", j=nsl)
                    ta_b = bass.AP(
                        ta[:].tensor, ta[:].offset,
                        [ta[:].ap[0], [0, nsl], ta[:].ap[1]],
                    )
                    tb_b = bass.AP(
                        tb[:].tensor, tb[:].offset,
                        [tb[:].ap[0], [0, nsl], tb[:].ap[1]],
                    )
                    nc.vector.tensor_mul(u3, x3, tb_b)
                    nc.vector.tensor_mul(o3, x3, ta_b)
                    us = ut[:]
                usw = us.rearrange("p (n two) -> p n two", two=2)[:, :, ::-1]
                os3 = os_.rearrange("p (n two) -> p n two", two=2)
                nc.vector.tensor_add(os3, os3, usw)

            row0 = 0
            for ci, nsl in enumerate(CHUNK_PLAN):
                first = ci == 0
                last = ci == len(CHUNK_PLAN) - 1
                cfree = nsl * FREE
                xoff = row0 * FREE
                rows = x_d[row0 * 128 : (row0 + nsl) * 128, :]
                xts = xbig[:, xoff : xoff + cfree]
                if first:
                    # split the first load so compute can start after 0.25 MiB
                    assert nsl == 1
                    h = cfree // 2
                    nc.sync.dma_start(xts[:, :h], rows[:, :h])
                    nc.sync.dma_start(xts[:, h:], rows[:, h:])
                else:
                    src = rows.rearrange("(j p) f -> p j f", j=nsl)
                    nc.sync.dma_start(
                        xts.rearrange("p (j f) -> p j f", j=nsl), src
                    )

                ut = upool.tile([128, cfree], f16, tag="u")
                ots = obig[:, xoff : xoff + cfree]
                orows = o_d[row0 * 128 : (row0 + nsl) * 128, :]

                if first or last:
                    # head and tail chunks in quarters. Head: quarter qi's
                    # compute depends only on the table pieces emitted
                    # before it (q0 gates on one 0.25 MB piece per table);
                    # the second table halves go to the sync ring after q1,
                    # whose compute only needs cols < hf. Tail: the final
                    # store is only 0.125 MB, shrinking the end-of-run
                    # barrier wait for it.
                    nparts = 4
                    h = cfree // nparts
                    for hi in range(nparts):
                        compute(xoff, ut, 1, hi * h, h)
                        if first and hi == 1:
                            # second table halves, after the head's
                            # below-hf computes in emission order
                            nc.sync.dma_start(tb[:, hf:], tb_d[:, hf:])
                            nc.sync.dma_start(ta[:, hf:], ta_d[:, hf:])
                        nc.scalar.dma_start(
                            orows[:, hi * h : (hi + 1) * h],
                            ots[:, hi * h : (hi + 1) * h],
                        )
                else:
                    compute(xoff, ut, nsl, 0, cfree)
                    dst = orows.rearrange("(j p) f -> p j f", j=nsl)
                    nc.scalar.dma_start(
                        dst, ots.rearrange("p (j f) -> p j f", j=nsl)
                    )
                row0 += nsl

    nc.compile()
    _prog_cache["nc"] = nc
    return nc


def _default_rotate_m(theta=10000.0):
    """Rebuild the reference's rotation buffer if the harness doesn't pass it."""
    half = D // 2
    try:  # replicate the reference's jax-f32 arithmetic exactly if possible
        import jax.numpy as jnp

        pos = np.asarray(jnp.arange(S, dtype=jnp.float32))
        inv_freq = np.asarray(
            theta ** (-(2.0 * jnp.arange(half, dtype=jnp.float32)) / D)
        )
        ang = np.asarray(pos[:, None] * inv_freq[None, :], dtype=np.float32)
        c, s = np.asarray(jnp.cos(ang)), np.asarray(jnp.sin(ang))
    except Exception:
        pos = np.arange(S, dtype=np.float32)
        exp = (-(2.0 * np.arange(half, dtype=np.float32)) / D).astype(np.float32)
        inv_freq = np.power(np.float32(theta), exp, dtype=np.float32)
        ang = (pos[:, None] * inv_freq[None, :]).astype(np.float32)
        c, s = np.cos(ang, dtype=np.float32), np.sin(ang, dtype=np.float32)
    idx = 2 * np.arange(half)
    r = np.zeros((S, D, D), dtype=np.float32)
    r[:, idx, idx] = c
    r[:, idx, idx + 1] = -s
    r[:, idx + 1, idx] = s
    r[:, idx + 1, idx + 1] = c
    return r


def _tables(token_positions, rotate_m):
    """Host-precompute the [128, FREE] fp16 A and B' tables (see docstring)."""
    if rotate_m is None:
        rotate_m = _default_rotate_m()
    r = np.asarray(rotate_m, dtype=np.float32)[np.asarray(token_positions)]
    idx = np.arange(D // 2) * 2
    a = r[:, idx, idx]            # x_even -> out_even
    b = r[:, idx, idx + 1]        # x_odd  -> out_even
    c = r[:, idx + 1, idx + 1]    # x_odd  -> out_odd
    d = r[:, idx + 1, idx]        # x_even -> out_odd
    A = np.empty((S, D), np.float32)
    A[:, 0::2] = a
    A[:, 1::2] = c
    Bp = np.empty((S, D), np.float32)
    Bp[:, 0::2] = d
    Bp[:, 1::2] = b
    return (
        np.ascontiguousarray(A.reshape(128, FREE)).astype(np.float16),
        np.ascontiguousarray(Bp.reshape(128, FREE)).astype(np.float16),
    )


def _in_maps(x, token_positions, rotate_m):
    ta, tb = _tables(token_positions, rotate_m)
    xs = np.asarray(x, dtype=np.float32).astype(np.float16).reshape(
        N_CORES, ROWS, FREE
    )
    xs = np.ascontiguousarray(xs)
    return [{"x": xs[i], "ta": ta, "tb": tb} for i in range(N_CORES)]


def _run(x, token_positions, rotate_m=None, trace=False, trace_cores=None):
    from concourse.bass_utils import run_bass_kernel_spmd

    nc = _build_program()
    in_maps = _in_maps(x, token_positions, rotate_m)
    res = run_bass_kernel_spmd(
        nc,
        in_maps,
        list(range(N_CORES)),
        trace=trace,
        trace_cores=trace_cores,
    )
    out = np.concatenate(
        [res.results[i]["out"].reshape(1, ROWS * FREE) for i in range(N_CORES)]
    ).reshape(B, H, S, D).astype(np.float32)
    return out, res


def kernel(x, token_positions, rotate_m=None, **_unused):
    out, _ = _run(x, token_positions, rotate_m, trace=False)
    return out
